# revision 62
# baseline (speedup 1.0000x reference)
"""GQA kernel for Trainium2, 8 NeuronCores.

Problem: x[1,4096,2048], H=16 heads, G=4 kv-groups, D=128, causal mask,
RoPE on q/k, out-proj. Sharding: 2 heads per core (core c -> heads 2c,2c+1,
kv-group c//2); out-proj sharded by output feature rows (core c -> dout
rows c*256..c*256+255, all 4096 positions).

Pipeline (all fp16 data path, fp32 PSUM accumulation):
  per q-chunk of 512:  project Q/K/V (K and V share one PSUM bank
  sequentially), RoPE on DVE, causal attention with scores one k-tile ahead
  of the PV matmuls (ACT exp latency hidden), softmax denominator via
  all-ones matmul (partition reduce+broadcast in one instruction) and
  1/d = exp(-ln d) on ACT. Context rows go out via grouped AllGathers
  that run on the CC stream while compute continues; all out-projections
  are emitted after attention(7) (any earlier placement delays the last
  chunk's gather 1:1), so the serial tail is just the last gather plus
  one chunk's out-proj.

Host supplies partition-major pre-layouts so every DMA moves >=8KB
contiguous per partition.
"""

import sys

for _p in ("/opt/trn_rl_repo",):
    if _p not in sys.path:
        sys.path.append(_p)

from contextlib import ExitStack

import numpy as np

import concourse.bass as bass
import concourse.tile as tile
from concourse import mybir
from concourse.bass_utils import run_bass_kernel_spmd

F32 = mybir.dt.float32
F16 = mybir.dt.float16
S = 4096
MAX_WAITS = 1  # walrus CoreV3 rejects instructions with more sync waits


def _split_sync_waits(nc, maxw=MAX_WAITS):
    """Hoist excess sem waits onto NOPs inserted before the instruction on
    the same engine queue (queue order makes this equivalent)."""
    from concourse import mybir as mb
    n = 0
    for bassbb in nc.bb_map.values():
        bb = bassbb.bb
        insts = list(bb.instructions)
        out = []
        changed = False
        for ins in insts:
            si = ins.sync_info
            if si is not None and si.on_wait and len(si.on_wait) > maxw:
                waits = list(si.on_wait)
                head, rest = waits[:-maxw], waits[-maxw:]
                while head:
                    chunk, head = head[:maxw], head[maxw:]
                    n += 1
                    nop = mb.InstNoOp(
                        name=f"I-ws{n}",
                        engine=ins.engine,
                        ins=[],
                        outs=[],
                        sync_info=mb.SyncInfo(on_wait=chunk, on_update=[]),
                    )
                    nc.register_instruction(nop)
                    out.append(nop)
                ins.sync_info = mb.SyncInfo(
                    on_wait=rest, on_update=list(si.on_update or []))
                changed = True
            out.append(ins)
        if changed:
            try:
                bb.instructions[:] = out
            except TypeError:
                bb.set_instructions(out)
    return n


DIN = 2048
D = 128
HPC = 2          # heads per core
DOUT_PC = HPC * D  # out-proj rows per core
NCORES = 8
QC = 512         # q-chunk (free dim per matmul)
NQ = S // QC     # 8 q-chunks
KT = 128         # k tile (partition dim)
NKIN = DIN // 128  # 16 contraction tiles for projections
NDT = DOUT_PC // 128  # 2 dout tiles per core
INV_SQRT_D = 1.0 / np.sqrt(D)
EXP_BIAS = -2.0  # keeps fp16 softmax sums well inside range; cancels in norm
# iteration -> chunks whose out-proj runs there (gathers known complete)
# all out-projs sit after attention(7) in the PE stream: any op emitted in
# an earlier iteration delays attention(7) -- and so the last gather -- 1:1
OP_SCHEDULE = {7: (0, 1, 2, 3, 4, 5)}


def build_nc():
    nc = bass.Bass(num_devices=NCORES)

    # partition-major pre-layouts (big contiguous runs per partition)
    xP = nc.dram_tensor("xP", [128, NQ, NKIN, QC], F16, kind="ExternalInput")
    wqP = nc.dram_tensor("wqP", [128, NKIN, HPC * D], F16, kind="ExternalInput")
    wkP = nc.dram_tensor("wkP", [128, NKIN, D], F16, kind="ExternalInput")
    wvP = nc.dram_tensor("wvP", [128, NKIN, D], F16, kind="ExternalInput")
    woP = nc.dram_tensor("woP", [128, NKIN, DOUT_PC], F16, kind="ExternalInput")
    cosT = nc.dram_tensor("cosT", [D, S], F16, kind="ExternalInput")
    sinT = nc.dram_tensor("sinT", [D, S], F16, kind="ExternalInput")
    outP = nc.dram_tensor("outP", [128, NDT, S], F32, kind="ExternalOutput")

    # exchange buffers (collectives can't touch I/O tensors). Early chunks
    # are gathered in pairs: each gather carries ~12us fixed cost on the CC
    # stream, and the stream is blocked ~75us at start by the NRT comm-init
    # barrier, so fewer early collectives means earlier availability.
    cc_in = nc.dram_tensor("cc_in", [NQ, HPC * D, QC], F16)
    GATHER_GROUPS = [(0, 2), (2, 4), (4, 6), (6, 7), (7, 8)]
    cc_gs = {}
    for a, b in GATHER_GROUPS:
        cc_gs[a, b] = nc.dram_tensor(f"cc_g{a}", [NCORES, b - a, HPC * D, QC],
                                     F16)
    group_of = {}
    for a, b in GATHER_GROUPS:
        for qc in range(a, b):
            group_of[qc] = (a, b)

    with ExitStack() as ctx:
        tc = ctx.enter_context(tile.TileContext(nc))

        res = ctx.enter_context(tc.tile_pool(name="res", bufs=1))
        # resident SBUF tensors
        qt = res.tile([128, HPC, S], F16, tag="qt")          # QT per head
        kt = res.tile([128, S], F16, tag="kt")               # KT (shared group)
        vt = res.tile([128, S // 128, D], F16, tag="vt")     # V as s-tiles
        wq_sb = res.tile([128, NKIN, HPC * D], F16, tag="wq")
        wk_sb = res.tile([128, NKIN, D], F16, tag="wk")
        wv_sb = res.tile([128, NKIN, D], F16, tag="wv")
        wo_sb = res.tile([128, NKIN, DOUT_PC], F16, tag="wo")
        cos_sb = res.tile([128, S], F16, tag="cos")
        sin_sb = res.tile([128, S], F16, tag="sin")
        ebias = res.tile([128, 1], F32, tag="ebias")         # exp bias const
        ones128 = res.tile([128, 128], F16, tag="ones128")   # partition reduce+bcast

        nc.vector.memset(ebias, EXP_BIAS)
        nc.vector.memset(ones128, 1.0)



        # ---------------- fused main loop ----------------
        with ExitStack() as p2:
            xpool = p2.enter_context(tc.tile_pool(name="xpool", bufs=2))
            rpool = p2.enter_context(tc.tile_pool(name="rope", bufs=3))
            wpool = p2.enter_context(tc.tile_pool(name="wpool", bufs=6))
            apool = p2.enter_context(tc.tile_pool(name="acc", bufs=2))
            npool = p2.enter_context(tc.tile_pool(name="norm", bufs=2))
            copool = p2.enter_context(tc.tile_pool(name="cout", bufs=2))
            cpool = p2.enter_context(tc.tile_pool(name="cpool", bufs=2))
            opool = p2.enter_context(tc.tile_pool(name="opool", bufs=2))
            # PSUM: 8 banks of [128, 2KB]:
            #   pq/po shared tag (2) + pkv (1) + ps (2) + pd (1) + pc (2)
            pq_pool = p2.enter_context(tc.tile_pool(name="pq", bufs=2, space="PSUM"))
            pkv_pool = p2.enter_context(tc.tile_pool(name="pkv", bufs=1, space="PSUM"))
            ps_pool = p2.enter_context(tc.tile_pool(name="ps", bufs=2, space="PSUM"))
            pc_pool = p2.enter_context(tc.tile_pool(name="pc", bufs=2, space="PSUM"))

            def load_xt(qc):
                t = xpool.tile([128, NKIN, QC], F16, tag="xt")
                nc.sync.dma_start(out=t, in_=xP[:, qc, :, :])
                return t

            def load_ccsb(qc):
                a, b = group_of[qc]
                cc_sb = cpool.tile([128, NKIN, QC], F16, tag="cc_sb",
                                   name="cc_sb")
                for src in range(NCORES):
                    nc.sync.dma_start(
                        out=cc_sb[:, src * HPC:(src + 1) * HPC, :],
                        in_=cc_gs[a, b][src, qc - a]
                            .rearrange("(t p) m -> p t m", p=128))
                return cc_sb

            def emit_outproj(qc, cc_sb):
                """out-proj of chunk qc (gathered OP_LAG chunks ago)."""
                for dt in range(NDT):
                    po = pq_pool.tile([128, QC], F32, tag="pq", name="po")
                    for ki in range(NKIN):
                        nc.tensor.matmul(
                            po, lhsT=wo_sb[:, ki, dt * 128:(dt + 1) * 128],
                            rhs=cc_sb[:, ki, :], start=(ki == 0),
                            stop=(ki == NKIN - 1))
                    ot = opool.tile([128, QC], F32, tag="ot", name="ot")
                    nc.vector.tensor_copy(ot, po)
                    nc.sync.dma_start(
                        out=outP[:, dt, qc * QC:(qc + 1) * QC], in_=ot)

            # first x-chunk split across the two hwdge queues so both halves
            # transfer in parallel; everything not needed immediately goes on
            # the scalar queue
            xt_next = xpool.tile([128, NKIN, QC], F16, tag="xt", name="xt0")
            nc.sync.dma_start(out=xt_next[:, 0:NKIN // 2, :],
                              in_=xP[:, 0, 0:NKIN // 2, :])
            nc.scalar.dma_start(out=xt_next[:, NKIN // 2:, :],
                                in_=xP[:, 0, NKIN // 2:, :])
            nc.sync.dma_start(out=wq_sb, in_=wqP[:, :, :])
            nc.sync.dma_start(out=wk_sb, in_=wkP[:, :, :])
            nc.sync.dma_start(out=wv_sb, in_=wvP[:, :, :])
            nc.scalar.dma_start(out=cos_sb, in_=cosT[:, :])
            nc.scalar.dma_start(out=sin_sb, in_=sinT[:, :])
            nc.scalar.dma_start(out=wo_sb, in_=woP[:, :, :])
            for qc in range(NQ):
                q0 = qc * QC
                # -------- projections --------
                xt = xt_next
                q2 = rpool.tile([128, HPC, QC], F16, tag="q2")
                for h in range(HPC):
                    pq = pq_pool.tile([128, QC], F32, tag="pq")
                    for ki in range(NKIN):
                        nc.tensor.matmul(
                            pq, lhsT=wq_sb[:, ki, h * D:(h + 1) * D],
                            rhs=xt[:, ki, :], start=(ki == 0),
                            stop=(ki == NKIN - 1))
                    nc.vector.tensor_copy(q2[:, h, :], pq)
                # K then V through one PSUM bank (freed by the k1 copy)
                pk = pkv_pool.tile([128, QC], F32, tag="pkv", name="pk")
                for ki in range(NKIN):
                    nc.tensor.matmul(pk, lhsT=wk_sb[:, ki, :], rhs=xt[:, ki, :],
                                     start=(ki == 0), stop=(ki == NKIN - 1))
                k1 = rpool.tile([128, QC], F16, tag="k1")
                nc.vector.tensor_copy(k1, pk)
                pvt = pkv_pool.tile([128, QC], F32, tag="pkv", name="pvt")
                for ki in range(NKIN):
                    nc.tensor.matmul(pvt, lhsT=wv_sb[:, ki, :], rhs=xt[:, ki, :],
                                     start=(ki == 0), stop=(ki == NKIN - 1))
                # prefetch next x-chunk while this chunk's attention runs
                if qc + 1 < NQ:
                    xt_next = load_xt(qc + 1)

                # VT -> V via DMA XBAR transpose (fp16, runs on DMA engines:
                # out[p, m, d] = in[d, m*128 + p], exactly vt's layout)
                vtT = rpool.tile([128, QC], F16, tag="vtT")
                nc.vector.tensor_copy(vtT, pvt)
                nc.sync.dma_start_transpose(
                    out=vt[:, qc * 4:(qc + 1) * 4, :], in_=vtT)

                # -------- RoPE (all-fp16 on DVE) --------
                cos_c = cos_sb[:, q0:q0 + QC]
                sin_c = sin_sb[:, q0:q0 + QC]

                def rope(dst, src):
                    rot = rpool.tile([128, QC], F16, tag="rot")
                    nc.vector.tensor_scalar_mul(rot[0:64, :], src[64:128, :], -1.0)
                    nc.vector.tensor_copy(rot[64:128, :], src[0:64, :])
                    nc.vector.tensor_mul(dst, src, cos_c)
                    nc.vector.tensor_mul(rot, rot, sin_c)
                    nc.vector.tensor_add(dst, dst, rot)

                for h in range(HPC):
                    rope(qt[:, h, q0:q0 + QC], q2[:, h, :])
                rope(kt[:, q0:q0 + QC], k1)

                # -------- attention for this q-chunk --------
                nk = (qc + 1) * 4
                LAG = 1
                pcs = [pc_pool.tile([128, QC], F32, tag="pc", name=f"pc{h}")
                       for h in range(HPC)]
                accs = [apool.tile([128, QC], F16, tag="acc", name=f"acc{h}")
                        for h in range(HPC)]
                wts = {}

                def emit_scores(h, ki):
                    k0 = ki * KT
                    # queries j < k0-q0 are fully masked for this k-tile:
                    # shrink every op to the live [off:QC] column range
                    off = max(0, k0 - q0)
                    ps = ps_pool.tile([128, QC], F32, tag="ps")
                    nc.tensor.matmul(ps[:, off:], lhsT=kt[:, k0:k0 + KT],
                                     rhs=qt[:, h, q0 + off:q0 + QC],
                                     start=True, stop=True)
                    wt = wpool.tile([128, QC], F16, tag="wt")
                    nc.scalar.activation(wt[:, off:], ps[:, off:],
                                         mybir.ActivationFunctionType.Exp,
                                         scale=INV_SQRT_D, bias=ebias)
                    if k0 + KT - 1 > q0:
                        # keep where (q0+off+j) - (k0+p) >= 0
                        nc.gpsimd.affine_select(
                            out=wt[:, off:], in_=wt[:, off:],
                            pattern=[[1, QC - off]],
                            compare_op=mybir.AluOpType.is_ge, fill=0.0,
                            base=q0 + off - k0, channel_multiplier=-1)
                    wts[(h, ki)] = (wt, off)

                def emit_pv(h, ki):
                    wt, off = wts.pop((h, ki))
                    nc.tensor.matmul(pcs[h][:, off:], lhsT=vt[:, ki, :],
                                     rhs=wt[:, off:],
                                     start=(ki == 0), stop=(ki == nk - 1))
                    if ki == 0:
                        nc.vector.tensor_copy(accs[h], wt)
                    else:
                        nc.vector.tensor_add(accs[h][:, off:], accs[h][:, off:],
                                             wt[:, off:])

                for ki in range(nk):
                    for h in range(HPC):
                        emit_scores(h, ki)
                    if ki >= LAG:
                        for h in range(HPC):
                            emit_pv(h, ki - LAG)
                for ki in range(nk - LAG, nk):
                    for h in range(HPC):
                        emit_pv(h, ki)
                for h in range(HPC):
                    # denominator: all-ones matmul reduces over partitions AND
                    # broadcasts the sum to every partition in one instruction
                    pd = ps_pool.tile([128, QC], F32, tag="pd", bufs=1)
                    nc.tensor.matmul(pd, lhsT=ones128, rhs=accs[h],
                                     start=True, stop=True)
                    # 1/d as exp(-ln(d)) on ACT
                    lg = npool.tile([128, QC], F32, tag="lg")
                    nc.scalar.activation(lg, pd,
                                         mybir.ActivationFunctionType.Ln,
                                         scale=1.0)
                    rec = npool.tile([128, QC], F16, tag="rec")
                    nc.scalar.activation(rec, lg,
                                         mybir.ActivationFunctionType.Exp,
                                         scale=-1.0)
                    cout = copool.tile([128, QC], F16, tag="cout")
                    nc.vector.tensor_mul(cout, pcs[h], rec)
                    nc.gpsimd.dma_start(
                        out=cc_in[qc, h * D:(h + 1) * D, :], in_=cout)

                # gather this group's context rows from all cores (runs on
                # the CC stream while the next chunks compute)
                for (a, b), t in cc_gs.items():
                    if b - 1 == qc:
                        nc.gpsimd.collective_compute(
                            "AllGather",
                            mybir.AluOpType.bypass,
                            replica_groups=[list(range(NCORES))],
                            ins=[cc_in[a:b]],
                            outs=[t[:, :, :, :]],
                        )

                # out-proj of earlier chunks whose gathers have completed
                for opc in OP_SCHEDULE.get(qc, ()):
                    emit_outproj(opc, load_ccsb(opc))

            for qc in (NQ - 2, NQ - 1):
                emit_outproj(qc, load_ccsb(qc))

    _split_sync_waits(nc)
    return nc


_NC_CACHE = None


def _get_nc():
    global _NC_CACHE
    if _NC_CACHE is None:
        _NC_CACHE = build_nc()
    return _NC_CACHE


def _pmajor(a2d):
    """[T*128, M] -> [128, T, M] with row t*128+p landing at [p, t]."""
    t = a2d.shape[0] // 128
    return np.ascontiguousarray(
        a2d.reshape(t, 128, a2d.shape[1]).transpose(1, 0, 2))


def _make_in_maps(x, cos, sin, Wq, Wk, Wv, Wo):
    xT = x.reshape(S, DIN).T.astype(np.float16)          # [DIN, S]
    xPm = _pmajor(xT)                                    # [128, 16, 4096]
    xP = np.ascontiguousarray(
        xPm.reshape(128, NKIN, NQ, QC).transpose(0, 2, 1, 3))
    cosT = np.ascontiguousarray(cos.T.astype(np.float16))
    sinT = np.ascontiguousarray(sin.T.astype(np.float16))
    in_maps = []
    for c in range(NCORES):
        g = c // 2
        in_maps.append({
            "xP": xP,
            "wqP": _pmajor(Wq[c * 256:(c + 1) * 256, :].T.astype(np.float16)),
            "wkP": _pmajor(Wk[g * 128:(g + 1) * 128, :].T.astype(np.float16)),
            "wvP": _pmajor(Wv[g * 128:(g + 1) * 128, :].T.astype(np.float16)),
            "woP": _pmajor(np.ascontiguousarray(
                Wo[c * 256:(c + 1) * 256, :]).T.astype(np.float16)),
            "cosT": cosT,
            "sinT": sinT,
        })
    return in_maps


def run(x, cos, sin, Wq, Wk, Wv, Wo, trace=False, tmpdir=None):
    nc = _get_nc()
    in_maps = _make_in_maps(x, cos, sin, Wq, Wk, Wv, Wo)
    res = run_bass_kernel_spmd(nc, in_maps, list(range(NCORES)), trace=trace,
                               tmpdir=tmpdir)
    out = np.empty((1, S, DIN), dtype=np.float32)
    for c in range(NCORES):
        op = res.results[c]["outP"]                      # [128, NDT, S]
        for dt in range(NDT):
            out[0, :, c * 256 + dt * 128:c * 256 + (dt + 1) * 128] = op[:, dt, :].T
    return out, res


def kernel(x, mask, cos, sin, Wq, Wk, Wv, Wo):
    out, _ = run(np.asarray(x, dtype=np.float32), np.asarray(cos, np.float32),
                 np.asarray(sin, np.float32), np.asarray(Wq, np.float32),
                 np.asarray(Wk, np.float32), np.asarray(Wv, np.float32),
                 np.asarray(Wo, np.float32))
    return out
